# revision 17
# baseline (speedup 1.0000x reference)
"""AttentionSubsample kernel for 8 trn2 NeuronCores.

Sharding: head-parallel (8 heads -> 8 cores). Each core runs its head through
kv/q projection, attention and hardswish, then computes a PARTIAL output
projection (512 out channels x its 32 v-channels) which is summed across
cores with a per-q-chunk ReduceScatter; core i keeps output rows 64i:64i+64,
so the final BatchNorm is purely local.

Key tricks vs the v1 kernel:
- Pipelined startup: x is DMA'd in 4 token chunks with the kv projection,
  bn_stats and the (raw-)v transpose trailing each chunk, so the kv-stat
  barrier lands right after the last DMA.
- BN folding: the k-side BatchNorm shift cancels inside the softmax (it is
  constant per query), so QK runs on RAW k with q~ = (s_k*s_q) q_raw +
  s_k*t_q.  The v-side BatchNorm is folded into the post-attention drain:
  out = (av * s_v) * (1/denom) + t_v via scalar_tensor_tensor.
- Attention phase leaves the Activation engine exp-only (84 exps of
  [128,1344]); the exp(bias) multiplies alternate between DVE and Pool.
- Softmax denominator from a ones-column in the AV stationary (33rd col).
- ReduceScatter of bf16 partial projections replaces the AllGather of hs
  (smaller payload, no cross-core BN).
"""

import numpy as np
import ml_dtypes

import concourse.bass as bass
import concourse.mybir as mybir
import concourse.tile as tile
from concourse import bacc
from contextlib import ExitStack
from concourse.bass_utils import run_bass_kernel_spmd

BF16 = mybir.dt.bfloat16
F32 = mybir.dt.float32
bf16 = ml_dtypes.bfloat16

B = 2
ROW, COL = 63, 84
ROW_, COL_ = 32, 42
N = ROW * COL            # 5292 kv tokens
NQ = ROW_ * COL_         # 1344 q tokens
NPAD = 5376              # 42*128 padded kv tokens
KT = NPAD // 128         # 42 k-tiles
QC = 448                 # q chunk
NQC = NQ // QC           # 3
CIN = 256
H = 8
KD = 16
DV = 32
HKV = KD + DV            # 48 per-head kv channels
KVP = 64                 # padded kv rows: k at 0:16, v at 32:64
OC = 64                  # per-core slice of the 512 output channels
GRP = 2                  # k-tiles per exp group
NGRP = KT // GRP         # 21
EPS = 1e-5
SCALE = KD ** -0.5
NCORES = 8
XCH = 4                  # x DMA chunks
XCT = NPAD // XCH        # 1344 tokens per chunk
PU = XCT // QC           # 448-wide proj units per chunk (3)
SW = 441                 # bn stats window (12 * 441 = 5292 real tokens)

LAST_EXEC_NS = None
_prog_cache = {}


def _build_program(debug=False):
    nc = bacc.Bacc(num_devices=NCORES)

    xT = nc.dram_tensor("xT", [B, 2, 128, NPAD], BF16, kind="ExternalInput")
    xsT = nc.dram_tensor("xsT", [B, 2, 128, NQ], BF16, kind="ExternalInput")
    wkvT = nc.dram_tensor("wkvT", [2, 128, KVP], BF16, kind="ExternalInput")
    wqT = nc.dram_tensor("wqT", [2, 128, KD], BF16, kind="ExternalInput")
    wpT = nc.dram_tensor("wpT", [DV, 4, 128], BF16, kind="ExternalInput")
    kv_gb = nc.dram_tensor("kv_gb", [KVP, 2], F32, kind="ExternalInput")
    q_gb = nc.dram_tensor("q_gb", [KD, 2], F32, kind="ExternalInput")
    p_gb = nc.dram_tensor("p_gb", [OC, 2], F32, kind="ExternalInput")
    ebT = nc.dram_tensor("ebT", [NQC, NGRP, 128, GRP * QC], BF16,
                         kind="ExternalInput")
    yT = nc.dram_tensor("yT", [OC, B * NQ], F32, kind="ExternalOutput")
    if debug:
        dbg = {
            "dbg_ykv": nc.dram_tensor("dbg_ykv", [KVP, B, NPAD], BF16,
                                      kind="ExternalOutput"),
            "dbg_qT": nc.dram_tensor("dbg_qT", [KD, B, NQ], BF16,
                                     kind="ExternalOutput"),
            "dbg_vaug": nc.dram_tensor("dbg_vaug", [128, B, KT, DV + 1], BF16,
                                       kind="ExternalOutput"),
            "dbg_hsT": nc.dram_tensor("dbg_hsT", [DV, B, NQ], BF16,
                                      kind="ExternalOutput"),
            "dbg_mvkv": nc.dram_tensor("dbg_mvkv", [KVP, 2], F32,
                                       kind="ExternalOutput"),
            "dbg_ypar": nc.dram_tensor("dbg_ypar", [NQC, 128, 4, B * QC], BF16,
                                       kind="ExternalOutput"),
            "dbg_yfin": nc.dram_tensor("dbg_yfin", [OC, B, NQ], BF16,
                                       kind="ExternalOutput"),
        }

    with ExitStack() as ctx:
        tc = ctx.enter_context(tile.TileContext(nc))
        const = ctx.enter_context(tc.tile_pool(name="const", bufs=1))
        big = ctx.enter_context(tc.tile_pool(name="big", bufs=1))
        spool = ctx.enter_context(tc.tile_pool(name="spool", bufs=4))
        ebpool = ctx.enter_context(tc.tile_pool(name="ebpool", bufs=4))
        small = ctx.enter_context(tc.tile_pool(name="small", bufs=4))
        drain = ctx.enter_context(tc.tile_pool(name="drain", bufs=3))
        ypp = ctx.enter_context(tc.tile_pool(name="ypp", bufs=2))
        psA = ctx.enter_context(tc.tile_pool(name="psA", bufs=2, space="PSUM"))
        psB = ctx.enter_context(tc.tile_pool(name="psB", bufs=2, space="PSUM"))
        psC = ctx.enter_context(tc.tile_pool(name="psC", bufs=1, space="PSUM"))
        dram = ctx.enter_context(tc.tile_pool(name="dram", bufs=4, space="DRAM"))

        mult = mybir.AluOpType.mult
        add = mybir.AluOpType.add
        amin = mybir.AluOpType.min
        amax = mybir.AluOpType.max
        Act = mybir.ActivationFunctionType

        # ---------------- weights + constants (SP queue) ----------------
        wkv_sb = const.tile([128, 2, KVP], BF16, tag="wkv")
        wq_sb = const.tile([128, 2, KD], BF16, tag="wq")
        wp_sb = const.tile([DV, 4, 128], BF16, tag="wp")
        for c in range(2):
            nc.sync.dma_start(out=wkv_sb[:, c, :], in_=wkvT[c])
            nc.sync.dma_start(out=wq_sb[:, c, :], in_=wqT[c])
        nc.sync.dma_start(out=wp_sb, in_=wpT[:, :, :])
        kvgb_sb = const.tile([KVP, 2], F32, tag="kvgb")
        qgb_sb = const.tile([KD, 2], F32, tag="qgb")
        pgb_sb = const.tile([OC, 2], F32, tag="pgb")
        nc.sync.dma_start(out=kvgb_sb, in_=kv_gb[:, :])
        nc.sync.dma_start(out=qgb_sb, in_=q_gb[:, :])
        nc.sync.dma_start(out=pgb_sb, in_=p_gb[:, :])
        eps_t = const.tile([128, 1], F32, tag="eps")
        nc.vector.memset(eps_t, EPS)
        ones1_t = const.tile([1, DV], F32, tag="ones1")
        nc.vector.memset(ones1_t, 1.0)

        # v_aug gets its ones column once; the raw-v transpose fills 0:DV.
        v_aug = big.tile([128, B, KT, DV + 1], BF16, tag="vaug")
        nc.gpsimd.memset(v_aug[:, :, :, DV:DV + 1], 1.0)

        # ---------------- pipelined x DMA + kv projection ----------------
        xt_sb = big.tile([128, B, 2, NPAD], BF16, tag="xt")
        xs_sb = big.tile([128, B, 2, NQ], BF16, tag="xs")
        y_kv = big.tile([KVP, B, NPAD], BF16, tag="ykv")
        y_q = big.tile([KD, B, NQ], BF16, tag="yq")
        st_kv = small.tile([KVP, 2 * 12, 6], F32, tag="st_kv")
        st_q = small.tile([KD, 2 * NQC, 6], F32, tag="st_q")

        def kv_chunk(ch):
            for b in range(B):
                for c in range(2):
                    nc.sync.dma_start(
                        out=xt_sb[:, b, c, bass.ts(ch, XCT)],
                        in_=xT[b, c, :, bass.ts(ch, XCT)])
            for b in range(B):
                for u in range(PU):
                    t = ch * PU + u
                    ps = psB.tile([KVP, QC], F32, tag="ps_av")
                    for c in range(2):
                        nc.tensor.matmul(ps, wkv_sb[:, c, :],
                                         xt_sb[:, b, c, bass.ts(t, QC)],
                                         start=(c == 0), stop=(c == 1))
                    if (t + b) % 2 == 0:
                        nc.scalar.copy(out=y_kv[:, b, bass.ts(t, QC)], in_=ps)
                    else:
                        nc.vector.tensor_copy(y_kv[:, b, bass.ts(t, QC)], ps)
                # stats windows fully covered by this chunk
                for w in range(3 * ch, 3 * ch + 3):
                    nc.vector.bn_stats(out=st_kv[:, b * 12 + w, :],
                                       in_=y_kv[:, b, bass.ds(w * SW, SW)])

        kv_chunk(0)
        kv_chunk(1)
        # xs lands while kv chunk 2 streams
        for b in range(B):
            for c in range(2):
                nc.sync.dma_start(out=xs_sb[:, b, c, :], in_=xsT[b, c])
        kv_chunk(2)
        # q projection between kv chunks so PE picks it up as soon as xs lands
        for b in range(B):
            for t in range(NQC):
                ps = psB.tile([KD, QC], F32, tag="ps_av")
                for c in range(2):
                    nc.tensor.matmul(ps, wq_sb[:, c, :],
                                     xs_sb[:, b, c, bass.ts(t, QC)],
                                     start=(c == 0), stop=(c == 1))
                nc.scalar.copy(out=y_q[:, b, bass.ts(t, QC)], in_=ps)
                nc.vector.bn_stats(out=st_q[:, b * NQC + t, :],
                                   in_=y_q[:, b, bass.ts(t, QC)])
        kv_chunk(3)
        # raw v -> token-major staging, then packed into v_aug next to the
        # ones column (BN folded into drain)
        vtd = big.tile([128, B, KT, DV], BF16, tag="vtd")
        for b in range(B):
            nc.scalar.dma_start_transpose(out=vtd[:, b, :, :],
                                          in_=y_kv[32:KVP, b, :])
            nc.gpsimd.tensor_copy(v_aug[:, b, :, 0:DV], vtd[:, b, :, :])

        # ---------------- batch-norm scale/shift ----------------
        def bn_scale_shift(mv, gb, P, name):
            s = small.tile([P, 1], F32, tag=f"s_{name}")
            t = small.tile([P, 1], F32, tag=f"t_{name}")
            nc.scalar.activation(out=s, in_=mv[:, 1:2], func=Act.Sqrt,
                                 bias=eps_t[0:P])
            nc.vector.reciprocal(out=s, in_=s)
            nc.vector.tensor_mul(s, s, gb[:, 0:1])
            nc.vector.tensor_mul(t, mv[:, 0:1], s)
            nc.vector.tensor_scalar(out=t, in0=t, scalar1=-1.0, scalar2=None,
                                    op0=mult)
            nc.vector.tensor_add(t, t, gb[:, 1:2])
            return s, t

        mv_kv = small.tile([KVP, 2], F32, tag="mv_kv")
        nc.vector.bn_aggr(out=mv_kv, in_=st_kv)
        s_kv, t_kv = bn_scale_shift(mv_kv, kvgb_sb, KVP, "kv")
        mv_q = small.tile([KD, 2], F32, tag="mv_q")
        nc.vector.bn_aggr(out=mv_q, in_=st_q)
        s_q, t_q = bn_scale_shift(mv_q, qgb_sb, KD, "q")

        # q~ = (s_k*s_q) . q_raw + s_k*t_q   (k-side BN shift cancels in
        # softmax; s_v/t_v folded into the drain below)
        a_q = small.tile([KD, 1], F32, tag="a_q")
        b_q = small.tile([KD, 1], F32, tag="b_q")
        nc.vector.tensor_mul(a_q, s_kv[0:KD], s_q)
        nc.vector.tensor_mul(b_q, s_kv[0:KD], t_q)
        qT = big.tile([KD, B, NQ], BF16, tag="qT")
        for b in range(B):
            nc.vector.tensor_scalar(out=qT[:, b, :], in0=y_q[:, b, :],
                                    scalar1=a_q, scalar2=b_q,
                                    op0=mult, op1=add)
        # v-side scale/shift moved to base partition 0 for the drain ops
        s_v0 = small.tile([DV, 1], F32, tag="s_v0")
        t_v0 = small.tile([DV, 1], F32, tag="t_v0")
        nc.sync.dma_start(out=s_v0, in_=s_kv[32:KVP])
        nc.sync.dma_start(out=t_v0, in_=t_kv[32:KVP])
        c3_v = small.tile([DV, 1], F32, tag="c3v")
        nc.vector.tensor_scalar(out=c3_v, in0=t_v0, scalar1=3.0,
                                scalar2=None, op0=add)

        # ---------------- attention + partial projection ----------------
        hsT = big.tile([DV, B, NQ], BF16, tag="hsT")
        y_bnc = dram.tile([NQC, 4, 128, B * QC], BF16, tag="y_bnc")
        y_sl = dram.tile([NQC, OC, B * QC], BF16, tag="y_sl")
        y_fin = big.tile([OC, B, NQ], BF16, tag="y_fin")
        y_out = big.tile([OC, B, NQ], F32, tag="y_out")

        def emit_proj(qc):
            # partial projection: y_par[o*128:(o+1)*128, (b,q)] over this
            # core's 32 v-channels; drains alternate DVE/Pool.
            ypar = ypp.tile([128, 4, B, QC], BF16, tag="ypar")
            k = 0
            for o in range(4):
                for b in range(B):
                    pp = psC.tile([128, 2, QC // 2], F32, tag="pp")
                    for h2 in range(2):
                        nc.tensor.matmul(
                            pp[:, h2, :], wp_sb[:, o, :],
                            hsT[:, b, bass.ds(qc * QC + h2 * (QC // 2),
                                              QC // 2)],
                            start=True, stop=True)
                        dst = ypar[:, o, b, bass.ts(h2, QC // 2)]
                        nc.vector.tensor_copy(dst, pp[:, h2, :])
                        k += 1
            nc.sync.dma_start(
                out=y_bnc[qc].rearrange("o p (b q) -> p o b q", b=B),
                in_=ypar)
            if debug:
                nc.sync.dma_start(
                    out=dbg["dbg_ypar"][qc].rearrange(
                        "p o (b q) -> p o b q", b=B),
                    in_=ypar)
            nc.gpsimd.collective_compute(
                "ReduceScatter", add,
                replica_groups=[list(range(NCORES))],
                ins=[y_bnc[qc].opt()],
                outs=[y_sl[qc].opt()])

        for qc in range(NQC):
            avs = []
            for _b in range(B):
                av_t = psB.tile([DV + 1, QC], F32, tag="ps_av")
                avs.append(av_t)
            for g in range(NGRP):
                eb = ebpool.tile([128, GRP, QC], BF16, tag="eb")
                nc.sync.dma_start(
                    out=eb,
                    in_=ebT[qc, g].rearrange("p (i q) -> p i q", i=GRP))
                if qc > 0 and g == 2:
                    emit_proj(qc - 1)
                for b in range(B):
                    qk = psA.tile([128, GRP, 512], F32, tag="qk")
                    for i in range(GRP):
                        j = g * GRP + i
                        nc.tensor.matmul(qk[:, i, 0:QC],
                                         y_kv[0:KD, b, bass.ts(j, 128)],
                                         qT[:, b, bass.ts(qc, QC)],
                                         start=True, stop=True)
                    sp = spool.tile([128, GRP, QC], BF16, tag="sp")
                    nc.scalar.activation(out=sp, in_=qk[:, :, 0:QC],
                                         func=Act.Exp, scale=SCALE)
                    if g % 3 == 1:
                        nc.gpsimd.tensor_mul(sp, sp, eb)
                    else:
                        nc.vector.tensor_mul(sp, sp, eb)
                    for i in range(GRP):
                        j = g * GRP + i
                        nc.tensor.matmul(avs[b], v_aug[:, b, j, :],
                                         sp[:, i, :],
                                         start=(j == 0), stop=(j == KT - 1),
                                         skip_group_check=True)
            for b in range(B):
                # drain: park accumulator, then out = (av*s_v)/denom + t_v,
                # hardswish, all DVE-side so ACT stays exp-only.
                av = drain.tile([DV + 1, QC], F32, tag="av_sb")
                nc.vector.tensor_copy(av, avs[b])
                rec = drain.tile([1, QC], F32, tag="rec")
                nc.vector.reciprocal(out=rec, in_=av[DV:DV + 1, :])
                recb = psC.tile([DV, QC], F32, tag="recb")
                nc.tensor.matmul(recb, ones1_t, rec, start=True, stop=True)
                xo = drain.tile([DV, QC], F32, tag="xo")
                nc.vector.scalar_tensor_tensor(
                    out=xo, in0=av[0:DV, :], scalar=s_v0,
                    in1=recb, op0=mult, op1=mult)
                r3 = drain.tile([DV, QC], F32, tag="r3")
                nc.vector.tensor_scalar(out=r3, in0=xo, scalar1=c3_v,
                                        scalar2=0.0, op0=add, op1=amax)
                nc.vector.tensor_scalar(out=r3, in0=r3, scalar1=6.0,
                                        scalar2=1.0 / 6.0, op0=amin, op1=mult)
                nc.vector.scalar_tensor_tensor(
                    out=hsT[:, b, bass.ts(qc, QC)], in0=xo,
                    scalar=t_v0, in1=r3, op0=add, op1=mult)
        emit_proj(NQC - 1)

        # ---------------- gather slices + final BN ----------------
        for qc in range(NQC):
            nc.sync.dma_start(
                out=y_fin[:, :, bass.ts(qc, QC)],
                in_=y_sl[qc].rearrange("o (b q) -> o b q", b=B))
        if debug:
            nc.sync.dma_start(out=dbg["dbg_ykv"][:, :, :], in_=y_kv)
            nc.sync.dma_start(out=dbg["dbg_qT"][:, :, :], in_=qT)
            nc.sync.dma_start(out=dbg["dbg_vaug"][:, :, :, :], in_=v_aug)
            nc.sync.dma_start(out=dbg["dbg_hsT"][:, :, :], in_=hsT)
            nc.sync.dma_start(out=dbg["dbg_mvkv"][:, :], in_=mv_kv)
            nc.sync.dma_start(out=dbg["dbg_yfin"][:, :, :], in_=y_fin)
        st_p = small.tile([OC, B * NQC, 6], F32, tag="st_p")
        for b in range(B):
            for t in range(NQC):
                nc.vector.bn_stats(out=st_p[:, b * NQC + t, :],
                                   in_=y_fin[:, b, bass.ts(t, QC)])
        mv_p = small.tile([OC, 2], F32, tag="mv_p")
        nc.vector.bn_aggr(out=mv_p, in_=st_p)
        s_p, t_p = bn_scale_shift(mv_p, pgb_sb, OC, "p")
        nc.vector.tensor_scalar(out=y_out[:, 0, :], in0=y_fin[:, 0, :],
                                scalar1=s_p, scalar2=t_p, op0=mult, op1=add)
        nc.gpsimd.tensor_scalar(out=y_out[:, 1, :], in0=y_fin[:, 1, :],
                                scalar1=s_p, scalar2=t_p, op0=mult, op1=add)
        nc.sync.dma_start(out=yT[:, :],
                          in_=y_out.rearrange("o b q -> o (b q)"))

    nc.finalize()
    return nc


def _prep_inputs(x, kv_w, kv_g, kv_b, q_w, q_g, q_b, proj_w, proj_g, proj_b,
                 bias_table, bias_idxs):
    """Host-side sharding/layout prep. Returns list of 8 per-core input maps."""
    x = np.asarray(x, np.float32)
    xt = np.zeros((B, 2, 128, NPAD), np.float32)
    xTt = x.transpose(0, 2, 1)  # (B, 256, N)
    xt[:, :, :, :N] = xTt.reshape(B, 2, 128, N)
    xt = xt.astype(bf16)
    xs = x.reshape(B, ROW, COL, CIN)[:, ::2, ::2].reshape(B, NQ, CIN)
    xst = xs.transpose(0, 2, 1).reshape(B, 2, 128, NQ).astype(bf16)

    # exp(bias) tables per head, padded-k zeroed, laid out (NQC, NGRP, 128, GRP*QC)
    rank2 = np.asarray(bias_idxs)[0].reshape(ROW, COL)  # (dr, dc) -> id
    table2 = np.asarray(bias_table, np.float32)[:, rank2]  # (H, 63, 84)
    eb2 = np.exp(table2)
    kk = np.arange(N)
    qq = np.arange(NQ)
    DRm = np.abs(kk[:, None] // COL - 2 * (qq[None, :] // COL_))
    DCm = np.abs(kk[:, None] % COL - 2 * (qq[None, :] % COL_))

    in_maps = []
    for h in range(H):
        ebf = np.zeros((NPAD, NQ), np.float32)
        ebf[:N] = eb2[h][DRm, DCm]
        ebl = (ebf.reshape(NGRP, GRP, 128, NQC, QC)
               .transpose(3, 0, 2, 1, 4)
               .reshape(NQC, NGRP, 128, GRP * QC)).astype(bf16)
        sl = slice(h * HKV, (h + 1) * HKV)
        slq = slice(h * KD, (h + 1) * KD)
        slo = slice(h * OC, (h + 1) * OC)
        wkv_pad = np.zeros((KVP, CIN), np.float32)
        wkv_pad[0:KD] = np.asarray(kv_w, np.float32)[sl][0:KD]
        wkv_pad[32:KVP] = np.asarray(kv_w, np.float32)[sl][KD:HKV]
        kvgb_pad = np.zeros((KVP, 2), np.float32)
        kvgb_pad[:, 0] = 1.0
        kvgb_pad[0:KD, 0] = np.asarray(kv_g, np.float32)[sl][0:KD]
        kvgb_pad[0:KD, 1] = np.asarray(kv_b, np.float32)[sl][0:KD]
        kvgb_pad[32:KVP, 0] = np.asarray(kv_g, np.float32)[sl][KD:HKV]
        kvgb_pad[32:KVP, 1] = np.asarray(kv_b, np.float32)[sl][KD:HKV]
        # W_p columns for this head's 32 v-channels, as 4 stationary tiles
        # (32 contract, 128 out rows)
        wp_h = np.asarray(proj_w, np.float32)[:, h * DV:(h + 1) * DV]  # (512, 32)
        wp_t = np.ascontiguousarray(
            wp_h.T.reshape(DV, 4, 128)).astype(bf16)
        in_maps.append({
            "xT": xt,
            "xsT": xst,
            "wkvT": np.ascontiguousarray(
                wkv_pad.T.reshape(2, 128, KVP)).astype(bf16),
            "wqT": np.ascontiguousarray(
                np.asarray(q_w, np.float32)[slq].T.reshape(2, 128, KD)
            ).astype(bf16),
            "wpT": wp_t,
            "kv_gb": np.ascontiguousarray(kvgb_pad),
            "q_gb": np.ascontiguousarray(np.stack(
                [np.asarray(q_g, np.float32)[slq],
                 np.asarray(q_b, np.float32)[slq]], axis=1)),
            "p_gb": np.ascontiguousarray(np.stack(
                [np.asarray(proj_g, np.float32)[slo],
                 np.asarray(proj_b, np.float32)[slo]], axis=1)),
            "ebT": ebl,
        })
    return in_maps


def kernel(x, kv_w, kv_g, kv_b, q_w, q_g, q_b, proj_w, proj_g, proj_b,
           bias_table, bias_idxs, _trace=False):
    global LAST_EXEC_NS
    if "nc" not in _prog_cache:
        _prog_cache["nc"] = _build_program()
    nc = _prog_cache["nc"]
    in_maps = _prep_inputs(x, kv_w, kv_g, kv_b, q_w, q_g, q_b,
                           proj_w, proj_g, proj_b, bias_table, bias_idxs)
    res = run_bass_kernel_spmd(nc, in_maps, core_ids=list(range(NCORES)),
                               trace=_trace)
    LAST_EXEC_NS = res.exec_time_ns
    yts = [np.asarray(r["yT"]) for r in res.results]  # each (OC, B*NQ)
    y = np.concatenate(yts, axis=0)                   # (512, B*NQ)
    return np.ascontiguousarray(
        y.T.reshape(B, NQ, H * OC).astype(np.float32))


# revision 19
# speedup vs baseline: 1.2953x; 1.2953x over previous
"""AttentionSubsample kernel for 8 trn2 NeuronCores.

Sharding: head-parallel (8 heads -> 8 cores). Each core runs its head through
kv/q projection, attention and hardswish, then computes a PARTIAL output
projection (512 out channels x its 32 v-channels) which is summed across
cores with a per-q-chunk ReduceScatter; core i keeps output rows 64i:64i+64,
so the final BatchNorm is purely local.

Key tricks vs the v1 kernel:
- Pipelined startup: x is DMA'd in 4 token chunks with the kv projection,
  bn_stats and the (raw-)v transpose trailing each chunk, so the kv-stat
  barrier lands right after the last DMA.
- BN folding: the k-side BatchNorm shift cancels inside the softmax (it is
  constant per query), so QK runs on RAW k with q~ = (s_k*s_q) q_raw +
  s_k*t_q.  The v-side BatchNorm is folded into the post-attention drain:
  out = (av * s_v) * (1/denom) + t_v via scalar_tensor_tensor.
- Attention phase leaves the Activation engine exp-only (84 exps of
  [128,1344]); the exp(bias) multiplies alternate between DVE and Pool.
- Softmax denominator from a ones-column in the AV stationary (33rd col).
- ReduceScatter of bf16 partial projections replaces the AllGather of hs
  (smaller payload, no cross-core BN).
"""

import numpy as np
import ml_dtypes

import concourse.bass as bass
import concourse.mybir as mybir
import concourse.tile as tile
from concourse import bacc
from contextlib import ExitStack
from concourse.bass_utils import run_bass_kernel_spmd

BF16 = mybir.dt.bfloat16
F32 = mybir.dt.float32
bf16 = ml_dtypes.bfloat16

B = 2
ROW, COL = 63, 84
ROW_, COL_ = 32, 42
N = ROW * COL            # 5292 kv tokens
NQ = ROW_ * COL_         # 1344 q tokens
NPAD = 5376              # 42*128 padded kv tokens
KT = NPAD // 128         # 42 k-tiles
QC = 448                 # q chunk
NQC = NQ // QC           # 3
CIN = 256
H = 8
KD = 16
DV = 32
HKV = KD + DV            # 48 per-head kv channels
KVP = 64                 # padded kv rows: k at 0:16, v at 32:64
OC = 64                  # per-core slice of the 512 output channels
GRP = 2                  # k-tiles per exp group
NGRP = KT // GRP         # 21
EPS = 1e-5
SCALE = KD ** -0.5
NCORES = 8
XCH = 4                  # x DMA chunks
XCT = NPAD // XCH        # 1344 tokens per chunk
PU = XCT // QC           # 448-wide proj units per chunk (3)
SW = 441                 # bn stats window (12 * 441 = 5292 real tokens)

LAST_EXEC_NS = None
_prog_cache = {}


def _build_program(debug=False):
    nc = bacc.Bacc(num_devices=NCORES)

    xT = nc.dram_tensor("xT", [B, 2, 128, NPAD], BF16, kind="ExternalInput")
    xsT = nc.dram_tensor("xsT", [B, 2, 128, NQ], BF16, kind="ExternalInput")
    wkvT = nc.dram_tensor("wkvT", [2, 128, KVP], BF16, kind="ExternalInput")
    wqT = nc.dram_tensor("wqT", [2, 128, KD], BF16, kind="ExternalInput")
    wpT = nc.dram_tensor("wpT", [DV, 4, 128], BF16, kind="ExternalInput")
    kv_gb = nc.dram_tensor("kv_gb", [KVP, 2], F32, kind="ExternalInput")
    q_gb = nc.dram_tensor("q_gb", [KD, 2], F32, kind="ExternalInput")
    p_gb = nc.dram_tensor("p_gb", [OC, 2], F32, kind="ExternalInput")
    ebT = nc.dram_tensor("ebT", [NQC, NGRP, 128, GRP * QC], BF16,
                         kind="ExternalInput")
    yT = nc.dram_tensor("yT", [OC, B * NQ], F32, kind="ExternalOutput")
    if debug:
        dbg = {
            "dbg_ykv": nc.dram_tensor("dbg_ykv", [KVP, B, NPAD], BF16,
                                      kind="ExternalOutput"),
            "dbg_qT": nc.dram_tensor("dbg_qT", [KD, B, NQ], BF16,
                                     kind="ExternalOutput"),
            "dbg_vaug": nc.dram_tensor("dbg_vaug", [128, B, KT, DV + 1], BF16,
                                       kind="ExternalOutput"),
            "dbg_hsT": nc.dram_tensor("dbg_hsT", [DV, B, NQ], BF16,
                                      kind="ExternalOutput"),
            "dbg_mvkv": nc.dram_tensor("dbg_mvkv", [KVP, 2], F32,
                                       kind="ExternalOutput"),
            "dbg_ypar": nc.dram_tensor("dbg_ypar", [NQC, 128, 4, B * QC], BF16,
                                       kind="ExternalOutput"),
            "dbg_yfin": nc.dram_tensor("dbg_yfin", [OC, B, NQ], BF16,
                                       kind="ExternalOutput"),
        }

    with ExitStack() as ctx:
        tc = ctx.enter_context(tile.TileContext(nc))
        const = ctx.enter_context(tc.tile_pool(name="const", bufs=1))
        big = ctx.enter_context(tc.tile_pool(name="big", bufs=1))
        spool = ctx.enter_context(tc.tile_pool(name="spool", bufs=4))
        ebpool = ctx.enter_context(tc.tile_pool(name="ebpool", bufs=4))
        small = ctx.enter_context(tc.tile_pool(name="small", bufs=4))
        drain = ctx.enter_context(tc.tile_pool(name="drain", bufs=3))
        ypp = ctx.enter_context(tc.tile_pool(name="ypp", bufs=2))
        psA = ctx.enter_context(tc.tile_pool(name="psA", bufs=2, space="PSUM"))
        psB = ctx.enter_context(tc.tile_pool(name="psB", bufs=2, space="PSUM"))
        psC = ctx.enter_context(tc.tile_pool(name="psC", bufs=1, space="PSUM"))
        dram = ctx.enter_context(tc.tile_pool(name="dram", bufs=4, space="DRAM"))

        mult = mybir.AluOpType.mult
        add = mybir.AluOpType.add
        amin = mybir.AluOpType.min
        amax = mybir.AluOpType.max
        Act = mybir.ActivationFunctionType

        # ---------------- weights + constants (SP queue) ----------------
        wkv_sb = const.tile([128, 2, KVP], BF16, tag="wkv")
        wq_sb = const.tile([128, 2, KD], BF16, tag="wq")
        wp_sb = const.tile([DV, 4, 128], BF16, tag="wp")
        for c in range(2):
            nc.sync.dma_start(out=wkv_sb[:, c, :], in_=wkvT[c])
            nc.sync.dma_start(out=wq_sb[:, c, :], in_=wqT[c])
        nc.sync.dma_start(out=wp_sb, in_=wpT[:, :, :])
        kvgb_sb = const.tile([KVP, 2], F32, tag="kvgb")
        qgb_sb = const.tile([KD, 2], F32, tag="qgb")
        pgb_sb = const.tile([OC, 2], F32, tag="pgb")
        nc.sync.dma_start(out=kvgb_sb, in_=kv_gb[:, :])
        nc.sync.dma_start(out=qgb_sb, in_=q_gb[:, :])
        nc.sync.dma_start(out=pgb_sb, in_=p_gb[:, :])
        eps_t = const.tile([128, 1], F32, tag="eps")
        nc.vector.memset(eps_t, EPS)
        ones1_t = const.tile([1, DV], F32, tag="ones1")
        nc.vector.memset(ones1_t, 1.0)

        # v_aug gets its ones column once; the raw-v transpose fills 0:DV.
        v_aug = big.tile([128, B, KT, DV + 1], BF16, tag="vaug")
        nc.gpsimd.memset(v_aug[:, :, :, DV:DV + 1], 1.0)

        # ---------------- pipelined x DMA + kv projection ----------------
        xt_sb = big.tile([128, B, 2, NPAD], BF16, tag="xt")
        xs_sb = big.tile([128, B, 2, NQ], BF16, tag="xs")
        y_kv = big.tile([KVP, B, NPAD], BF16, tag="ykv")
        y_q = big.tile([KD, B, NQ], BF16, tag="yq")
        st_kv = small.tile([KVP, 2 * 12, 6], F32, tag="st_kv")
        st_q = small.tile([KD, 2 * NQC, 6], F32, tag="st_q")

        def kv_chunk(ch):
            for b in range(B):
                for c in range(2):
                    nc.sync.dma_start(
                        out=xt_sb[:, b, c, bass.ts(ch, XCT)],
                        in_=xT[b, c, :, bass.ts(ch, XCT)])
            for b in range(B):
                for u in range(PU):
                    t = ch * PU + u
                    ps = psB.tile([KVP, QC], F32, tag="ps_av")
                    for c in range(2):
                        nc.tensor.matmul(ps, wkv_sb[:, c, :],
                                         xt_sb[:, b, c, bass.ts(t, QC)],
                                         start=(c == 0), stop=(c == 1))
                    if (t + b) % 2 == 0:
                        nc.scalar.copy(out=y_kv[:, b, bass.ts(t, QC)], in_=ps)
                    else:
                        nc.vector.tensor_copy(y_kv[:, b, bass.ts(t, QC)], ps)
                # stats windows fully covered by this chunk
                for w in range(3 * ch, 3 * ch + 3):
                    nc.vector.bn_stats(out=st_kv[:, b * 12 + w, :],
                                       in_=y_kv[:, b, bass.ds(w * SW, SW)])

        kv_chunk(0)
        kv_chunk(1)
        # xs lands while kv chunk 2 streams
        for b in range(B):
            for c in range(2):
                nc.sync.dma_start(out=xs_sb[:, b, c, :], in_=xsT[b, c])
        kv_chunk(2)
        # q projection between kv chunks so PE picks it up as soon as xs lands
        for b in range(B):
            for t in range(NQC):
                ps = psB.tile([KD, QC], F32, tag="ps_av")
                for c in range(2):
                    nc.tensor.matmul(ps, wq_sb[:, c, :],
                                     xs_sb[:, b, c, bass.ts(t, QC)],
                                     start=(c == 0), stop=(c == 1))
                nc.scalar.copy(out=y_q[:, b, bass.ts(t, QC)], in_=ps)
                nc.vector.bn_stats(out=st_q[:, b * NQC + t, :],
                                   in_=y_q[:, b, bass.ts(t, QC)])
        kv_chunk(3)
        # raw v -> token-major staging, then packed into v_aug next to the
        # ones column (BN folded into drain)
        vtd = big.tile([128, B, KT, DV], BF16, tag="vtd")
        for b in range(B):
            nc.scalar.dma_start_transpose(out=vtd[:, b, :, :],
                                          in_=y_kv[32:KVP, b, :])
            nc.gpsimd.tensor_copy(v_aug[:, b, :, 0:DV], vtd[:, b, :, :])

        # ---------------- batch-norm scale/shift ----------------
        def bn_scale_shift(mv, gb, P, name):
            s = small.tile([P, 1], F32, tag=f"s_{name}")
            t = small.tile([P, 1], F32, tag=f"t_{name}")
            nc.scalar.activation(out=s, in_=mv[:, 1:2], func=Act.Sqrt,
                                 bias=eps_t[0:P])
            nc.vector.reciprocal(out=s, in_=s)
            nc.vector.tensor_mul(s, s, gb[:, 0:1])
            nc.vector.tensor_mul(t, mv[:, 0:1], s)
            nc.vector.tensor_scalar(out=t, in0=t, scalar1=-1.0, scalar2=None,
                                    op0=mult)
            nc.vector.tensor_add(t, t, gb[:, 1:2])
            return s, t

        mv_kv = small.tile([KVP, 2], F32, tag="mv_kv")
        nc.vector.bn_aggr(out=mv_kv, in_=st_kv)
        s_kv, t_kv = bn_scale_shift(mv_kv, kvgb_sb, KVP, "kv")
        mv_q = small.tile([KD, 2], F32, tag="mv_q")
        nc.vector.bn_aggr(out=mv_q, in_=st_q)
        s_q, t_q = bn_scale_shift(mv_q, qgb_sb, KD, "q")

        # q~ = (s_k*s_q) . q_raw + s_k*t_q   (k-side BN shift cancels in
        # softmax; s_v/t_v folded into the drain below)
        a_q = small.tile([KD, 1], F32, tag="a_q")
        b_q = small.tile([KD, 1], F32, tag="b_q")
        nc.vector.tensor_mul(a_q, s_kv[0:KD], s_q)
        nc.vector.tensor_mul(b_q, s_kv[0:KD], t_q)
        qT = big.tile([KD, B, NQ], BF16, tag="qT")
        for b in range(B):
            nc.vector.tensor_scalar(out=qT[:, b, :], in0=y_q[:, b, :],
                                    scalar1=a_q, scalar2=b_q,
                                    op0=mult, op1=add)
        # v-side scale/shift moved to base partition 0 for the drain ops
        s_v0 = small.tile([DV, 1], F32, tag="s_v0")
        t_v0 = small.tile([DV, 1], F32, tag="t_v0")
        nc.sync.dma_start(out=s_v0, in_=s_kv[32:KVP])
        nc.sync.dma_start(out=t_v0, in_=t_kv[32:KVP])
        c3_v = small.tile([DV, 1], F32, tag="c3v")
        nc.vector.tensor_scalar(out=c3_v, in0=t_v0, scalar1=3.0,
                                scalar2=None, op0=add)

        # ---------------- attention + partial projection ----------------
        hsT = big.tile([DV, B, NQ], BF16, tag="hsT")
        y_bnc = dram.tile([NQC, 4, 128, B * QC], BF16, tag="y_bnc")
        y_sl = dram.tile([NQC, OC, B * QC], BF16, tag="y_sl")
        y_fin = big.tile([OC, B, NQ], BF16, tag="y_fin")
        y_out = big.tile([OC, B, NQ], F32, tag="y_out")

        def emit_proj(qc):
            # partial projection: y_par[o*128:(o+1)*128, (b,q)] over this
            # core's 32 v-channels; drains alternate DVE/Pool.
            ypar = ypp.tile([128, 4, B, QC], BF16, tag="ypar")
            k = 0
            for o in range(4):
                for b in range(B):
                    pp = psC.tile([128, 2, QC // 2], F32, tag="pp")
                    for h2 in range(2):
                        nc.tensor.matmul(
                            pp[:, h2, :], wp_sb[:, o, :],
                            hsT[:, b, bass.ds(qc * QC + h2 * (QC // 2),
                                              QC // 2)],
                            start=True, stop=True)
                        dst = ypar[:, o, b, bass.ts(h2, QC // 2)]
                        nc.vector.tensor_copy(dst, pp[:, h2, :])
                        k += 1
            nc.sync.dma_start(
                out=y_bnc[qc].rearrange("o p (b q) -> p o b q", b=B),
                in_=ypar)
            if debug:
                nc.sync.dma_start(
                    out=dbg["dbg_ypar"][qc].rearrange(
                        "p o (b q) -> p o b q", b=B),
                    in_=ypar)
            nc.gpsimd.collective_compute(
                "ReduceScatter", add,
                replica_groups=[list(range(NCORES))],
                ins=[y_bnc[qc].opt()],
                outs=[y_sl[qc].opt()])

        for qc in range(NQC):
            avs = []
            for _b in range(B):
                av_t = psB.tile([DV + 1, QC], F32, tag="ps_av")
                avs.append(av_t)
            for g in range(NGRP):
                eb = ebpool.tile([128, GRP, QC], BF16, tag="eb")
                nc.sync.dma_start(
                    out=eb,
                    in_=ebT[qc, g].rearrange("p (i q) -> p i q", i=GRP))
                if qc > 0 and g == 2:
                    emit_proj(qc - 1)
                for b in range(B):
                    qk = psA.tile([128, GRP, 512], F32, tag="qk")
                    for i in range(GRP):
                        j = g * GRP + i
                        nc.tensor.matmul(qk[:, i, 0:QC],
                                         y_kv[0:KD, b, bass.ts(j, 128)],
                                         qT[:, b, bass.ts(qc, QC)],
                                         start=True, stop=True)
                    sp = spool.tile([128, GRP, QC], BF16, tag="sp")
                    nc.scalar.activation(out=sp, in_=qk[:, :, 0:QC],
                                         func=Act.Exp, scale=SCALE)
                    nc.vector.tensor_mul(sp, sp, eb)
                    for i in range(GRP):
                        j = g * GRP + i
                        nc.tensor.matmul(avs[b], v_aug[:, b, j, :],
                                         sp[:, i, :],
                                         start=(j == 0), stop=(j == KT - 1),
                                         skip_group_check=True)
            for b in range(B):
                # drain: park accumulator, then out = (av*s_v)/denom + t_v,
                # hardswish, all DVE-side so ACT stays exp-only.
                av = drain.tile([DV + 1, QC], F32, tag="av_sb")
                nc.vector.tensor_copy(av, avs[b])
                rec = drain.tile([1, QC], F32, tag="rec")
                nc.vector.reciprocal(out=rec, in_=av[DV:DV + 1, :])
                recb = psC.tile([DV, QC], F32, tag="recb")
                nc.tensor.matmul(recb, ones1_t, rec, start=True, stop=True)
                xo = drain.tile([DV, QC], F32, tag="xo")
                nc.vector.scalar_tensor_tensor(
                    out=xo, in0=av[0:DV, :], scalar=s_v0,
                    in1=recb, op0=mult, op1=mult)
                r3 = drain.tile([DV, QC], F32, tag="r3")
                nc.vector.tensor_scalar(out=r3, in0=xo, scalar1=c3_v,
                                        scalar2=0.0, op0=add, op1=amax)
                nc.vector.tensor_scalar(out=r3, in0=r3, scalar1=6.0,
                                        scalar2=1.0 / 6.0, op0=amin, op1=mult)
                nc.vector.scalar_tensor_tensor(
                    out=hsT[:, b, bass.ts(qc, QC)], in0=xo,
                    scalar=t_v0, in1=r3, op0=add, op1=mult)
        emit_proj(NQC - 1)

        # ---------------- gather slices + final BN ----------------
        # tile_wait_until keeps the scheduler from hoisting these (they wait
        # on the ReduceScatters) into the attention phase, which would
        # head-of-line-block the SP/DVE queues.
        ctx.enter_context(tc.tile_wait_until(0.25))
        for qc in range(NQC):
            nc.sync.dma_start(
                out=y_fin[:, :, bass.ts(qc, QC)],
                in_=y_sl[qc].rearrange("o (b q) -> o b q", b=B))
        if debug:
            nc.sync.dma_start(out=dbg["dbg_ykv"][:, :, :], in_=y_kv)
            nc.sync.dma_start(out=dbg["dbg_qT"][:, :, :], in_=qT)
            nc.sync.dma_start(out=dbg["dbg_vaug"][:, :, :, :], in_=v_aug)
            nc.sync.dma_start(out=dbg["dbg_hsT"][:, :, :], in_=hsT)
            nc.sync.dma_start(out=dbg["dbg_mvkv"][:, :], in_=mv_kv)
            nc.sync.dma_start(out=dbg["dbg_yfin"][:, :, :], in_=y_fin)
        st_p = small.tile([OC, B * NQC, 6], F32, tag="st_p")
        for b in range(B):
            for t in range(NQC):
                nc.vector.bn_stats(out=st_p[:, b * NQC + t, :],
                                   in_=y_fin[:, b, bass.ts(t, QC)])
        mv_p = small.tile([OC, 2], F32, tag="mv_p")
        nc.vector.bn_aggr(out=mv_p, in_=st_p)
        s_p, t_p = bn_scale_shift(mv_p, pgb_sb, OC, "p")
        nc.vector.tensor_scalar(out=y_out[:, 0, :], in0=y_fin[:, 0, :],
                                scalar1=s_p, scalar2=t_p, op0=mult, op1=add)
        nc.vector.tensor_scalar(out=y_out[:, 1, :], in0=y_fin[:, 1, :],
                                scalar1=s_p, scalar2=t_p, op0=mult, op1=add)
        nc.sync.dma_start(out=yT[:, :],
                          in_=y_out.rearrange("o b q -> o (b q)"))

    nc.finalize()
    return nc


def _prep_inputs(x, kv_w, kv_g, kv_b, q_w, q_g, q_b, proj_w, proj_g, proj_b,
                 bias_table, bias_idxs):
    """Host-side sharding/layout prep. Returns list of 8 per-core input maps."""
    x = np.asarray(x, np.float32)
    xt = np.zeros((B, 2, 128, NPAD), np.float32)
    xTt = x.transpose(0, 2, 1)  # (B, 256, N)
    xt[:, :, :, :N] = xTt.reshape(B, 2, 128, N)
    xt = xt.astype(bf16)
    xs = x.reshape(B, ROW, COL, CIN)[:, ::2, ::2].reshape(B, NQ, CIN)
    xst = xs.transpose(0, 2, 1).reshape(B, 2, 128, NQ).astype(bf16)

    # exp(bias) tables per head, padded-k zeroed, laid out (NQC, NGRP, 128, GRP*QC)
    rank2 = np.asarray(bias_idxs)[0].reshape(ROW, COL)  # (dr, dc) -> id
    table2 = np.asarray(bias_table, np.float32)[:, rank2]  # (H, 63, 84)
    eb2 = np.exp(table2)
    kk = np.arange(N)
    qq = np.arange(NQ)
    DRm = np.abs(kk[:, None] // COL - 2 * (qq[None, :] // COL_))
    DCm = np.abs(kk[:, None] % COL - 2 * (qq[None, :] % COL_))

    in_maps = []
    for h in range(H):
        ebf = np.zeros((NPAD, NQ), np.float32)
        ebf[:N] = eb2[h][DRm, DCm]
        ebl = (ebf.reshape(NGRP, GRP, 128, NQC, QC)
               .transpose(3, 0, 2, 1, 4)
               .reshape(NQC, NGRP, 128, GRP * QC)).astype(bf16)
        sl = slice(h * HKV, (h + 1) * HKV)
        slq = slice(h * KD, (h + 1) * KD)
        slo = slice(h * OC, (h + 1) * OC)
        wkv_pad = np.zeros((KVP, CIN), np.float32)
        wkv_pad[0:KD] = np.asarray(kv_w, np.float32)[sl][0:KD]
        wkv_pad[32:KVP] = np.asarray(kv_w, np.float32)[sl][KD:HKV]
        kvgb_pad = np.zeros((KVP, 2), np.float32)
        kvgb_pad[:, 0] = 1.0
        kvgb_pad[0:KD, 0] = np.asarray(kv_g, np.float32)[sl][0:KD]
        kvgb_pad[0:KD, 1] = np.asarray(kv_b, np.float32)[sl][0:KD]
        kvgb_pad[32:KVP, 0] = np.asarray(kv_g, np.float32)[sl][KD:HKV]
        kvgb_pad[32:KVP, 1] = np.asarray(kv_b, np.float32)[sl][KD:HKV]
        # W_p columns for this head's 32 v-channels, as 4 stationary tiles
        # (32 contract, 128 out rows)
        wp_h = np.asarray(proj_w, np.float32)[:, h * DV:(h + 1) * DV]  # (512, 32)
        wp_t = np.ascontiguousarray(
            wp_h.T.reshape(DV, 4, 128)).astype(bf16)
        in_maps.append({
            "xT": xt,
            "xsT": xst,
            "wkvT": np.ascontiguousarray(
                wkv_pad.T.reshape(2, 128, KVP)).astype(bf16),
            "wqT": np.ascontiguousarray(
                np.asarray(q_w, np.float32)[slq].T.reshape(2, 128, KD)
            ).astype(bf16),
            "wpT": wp_t,
            "kv_gb": np.ascontiguousarray(kvgb_pad),
            "q_gb": np.ascontiguousarray(np.stack(
                [np.asarray(q_g, np.float32)[slq],
                 np.asarray(q_b, np.float32)[slq]], axis=1)),
            "p_gb": np.ascontiguousarray(np.stack(
                [np.asarray(proj_g, np.float32)[slo],
                 np.asarray(proj_b, np.float32)[slo]], axis=1)),
            "ebT": ebl,
        })
    return in_maps


def kernel(x, kv_w, kv_g, kv_b, q_w, q_g, q_b, proj_w, proj_g, proj_b,
           bias_table, bias_idxs, _trace=False):
    global LAST_EXEC_NS
    if "nc" not in _prog_cache:
        _prog_cache["nc"] = _build_program()
    nc = _prog_cache["nc"]
    in_maps = _prep_inputs(x, kv_w, kv_g, kv_b, q_w, q_g, q_b,
                           proj_w, proj_g, proj_b, bias_table, bias_idxs)
    res = run_bass_kernel_spmd(nc, in_maps, core_ids=list(range(NCORES)),
                               trace=_trace)
    LAST_EXEC_NS = res.exec_time_ns
    yts = [np.asarray(r["yT"]) for r in res.results]  # each (OC, B*NQ)
    y = np.concatenate(yts, axis=0)                   # (512, B*NQ)
    return np.ascontiguousarray(
        y.T.reshape(B, NQ, H * OC).astype(np.float32))
